# revision 1
# baseline (speedup 1.0000x reference)
"""Trainium2 Bass kernel for a contextual loss (cosine-distance softmin loss).

Math (per batch b):
  mu_c      = mean_n Y[b,c,n]
  xc = X-mu, yc = Y-mu                      (centered, [C,N])
  G[i,j]    = <xc_i, yc_j>                  (K=C=64 matmul)
  s[i,j]    = rx_i * ry_j * G[i,j]          (cosine similarity; rx/ry = 1/norms)
  dmin_i    = 1 - max_j s[i,j]
  a_i       = 1 / (H * (dmin_i + EPS_MIN))
  CX_i      = max_j A / sum_j A = 1 / sum_j exp(a_i*(s_ij - smax_i))
  loss_b    = -log(mean_i CX_i)

Sharding: 8 cores = 4 batches x 2 row-halves. Each core gets its full-batch
Y [64,4096] and its half of X's columns [64,2048], computes
S'_i = sum_j exp(...) for its 2048 rows, returns S' as [128,16]
(partition p, chunk k  <->  row k*128+p). Host reduces to the [4] loss.

On-device pipeline per 128-row chunk:
  PE   : 8 matmuls (f32r, K=64, N=512) -> PSUM [128,2048] x2
  DVE  : tensor_tensor_reduce fuses (G * ry_bcast) -> SBUF rowbuf copy
         with a running row-max (accum), then tiny per-row scalar chain
  ACT  : exp(scale*q + bias) with per-partition scale/bias and
         accumulated row-sum (accum_out) -> S' contributions
"""

import math

import numpy as np

import concourse.bacc as bacc
import concourse.mybir as mybir
from concourse.dve_ops import TENSOR_MASK_REDUCE
from concourse.bass_utils import run_bass_kernel_spmd
from concourse.mybir import ActivationFunctionType as AF, AluOpType as OP, AxisListType
from concourse.tile import TileContext

F32 = mybir.dt.float32
F32R = mybir.dt.float32r

B, C, N = 4, 64, 4096          # batch, channels, spatial (64*64)
NX = N // 2                    # rows per core (half batch)
CH = NX // 128                 # 16 chunks of 128 rows
HALF = N // 2                  # column half processed per DVE op
H_BAND = 5.0
EPS_MIN = 1e-3
LN02 = math.log(0.2)           # fold the 1/H into rx via exp(... + ln(1/H))

_NC_CACHE = {}


def build_nc():
    nc = bacc.Bacc("TRN2", target_bir_lowering=False, debug=False, num_devices=8)
    x_d = nc.dram_tensor("Xh", [C, NX], F32, kind="ExternalInput")
    y_d = nc.dram_tensor("Yb", [C, N], F32, kind="ExternalInput")
    out_d = nc.dram_tensor("out", [128, CH], F32, kind="ExternalOutput")

    with TileContext(nc) as tc:
        with (
            tc.tile_pool(name="persist", bufs=1) as persist,
            tc.tile_pool(name="mm", bufs=2, space="PSUM") as mmpool,
            tc.tile_pool(name="rb", bufs=4) as rbpool,
            tc.tile_pool(name="small", bufs=4) as small,
            tc.tile_pool(name="escr", bufs=2) as escrpool,
        ):
            # ---------------- load inputs ----------------
            y_sb = persist.tile([C, N], F32)
            nc.sync.dma_start(out=y_sb[:], in_=y_d[:])
            x_sb = persist.tile([C, NX], F32)
            nc.sync.dma_start(out=x_sb[:], in_=x_d[:])

            ones_f = persist.tile([C, 128], F32)
            nc.vector.memset(ones_f[:], 1.0)
            ones_w = persist.tile([C, 128], F32R)
            nc.vector.tensor_copy(ones_w[:], ones_f[:])
            ones1 = persist.tile([C, 2], F32R)
            nc.vector.tensor_copy(ones1[:], ones_f[:, 0:2])

            # ---------------- center by Y's spatial mean ----------------
            musum = small.tile([C, 1], F32, tag="musum")
            nc.vector.reduce_sum(out=musum[:], in_=y_sb[:], axis=AxisListType.X)
            mu = small.tile([C, 1], F32, tag="mu")
            nc.vector.tensor_scalar_mul(mu[:], musum[:], 1.0 / N)

            ycen = persist.tile([C, N], F32)
            nc.vector.tensor_scalar(ycen[:], y_sb[:], mu[:], None, OP.subtract)
            xcen = persist.tile([C, NX], F32R)
            nc.vector.tensor_scalar(xcen[:], x_sb[:], mu[:], None, OP.subtract)

            ysq = persist.tile([C, N], F32R)
            nc.scalar.activation(ysq[:], ycen[:], AF.Square)
            xsq = persist.tile([C, NX], F32R)
            nc.scalar.activation(xsq[:], xcen[:], AF.Square)

            # ---------------- ry broadcast [128, N] ----------------
            # ones[64,128].T @ ysq[64,512] = column sums of ysq, replicated
            # down all 128 partitions.  ry = 1/sqrt(ny2) done as exp(-.5*ln).
            ry_bc = persist.tile([128, N], F32)
            for h in range(2):
                ps = mmpool.tile([128, HALF], F32, tag="mm")
                for j in range(4):
                    c0 = h * HALF + j * 512
                    nc.tensor.matmul(
                        ps[:, j * 512:(j + 1) * 512],
                        lhsT=ones_w[:],
                        rhs=ysq[:, c0:c0 + 512],
                        start=True, stop=True,
                    )
                tln = escrpool.tile([128, HALF], F32, tag="escr")
                nc.scalar.activation(tln[:], ps[:], AF.Ln)
                nc.scalar.activation(
                    ry_bc[:, h * HALF:(h + 1) * HALF], tln[:], AF.Exp, scale=-0.5
                )

            # ---------------- rx5 = 0.2 * 1/sqrt(nx2)  [128, CH] ----------------
            # xsq[64,128chunk].T @ ones[64,1] = per-row ||xc_i||^2 in
            # [128 rows, chunk] layout.
            nx2 = mmpool.tile([128, 2 * CH], F32, tag="mm")
            for k in range(CH):
                nc.tensor.matmul(
                    nx2[:, 2 * k:2 * k + 2],
                    lhsT=xsq[:, k * 128:(k + 1) * 128],
                    rhs=ones1[:],
                    start=True, stop=True,
                )
            yhat = persist.tile([C, N], F32R)
            nc.vector.tensor_tensor(yhat[:], ycen[:], ry_bc[:C, :], OP.mult)
            c3big = persist.tile([128, 1], F32)
            nc.vector.memset(c3big[:], 1.0e9)

            tn = small.tile([128, CH], F32, tag="tn")
            nc.scalar.activation(
                tn[:], nx2[:].rearrange("p (k two) -> p k two", two=2)[:, :, 0], AF.Ln
            )
            ln02 = persist.tile([128, 1], F32)
            nc.vector.memset(ln02[:], LN02)
            rx5 = persist.tile([128, CH], F32)
            nc.scalar.activation(rx5[:], tn[:], AF.Exp, bias=ln02[:], scale=-0.5)

            # ---------------- main loop ----------------
            ssums = persist.tile([128, 2 * CH], F32)
            for k in range(CH):
                lhs = xcen[:, k * 128:(k + 1) * 128]
                pm = small.tile([128, 2], F32, tag="pm")
                rbs = []
                for h in range(2):
                    ps = mmpool.tile([128, HALF], F32, tag="mm")
                    for j in range(4):
                        c0 = h * HALF + j * 512
                        nc.tensor.matmul(
                            ps[:, j * 512:(j + 1) * 512],
                            lhsT=lhs,
                            rhs=yhat[:, c0:c0 + 512],
                            start=True, stop=True,
                        )
                    rb = rbpool.tile([128, HALF], F32, tag="rb")
                    init = -3.0e38 if h == 0 else pm[:, 0:1]
                    # rb = copy(ps); pm[:,h] = max(row-max(rb), init)
                    # (custom-DVE mask-reduce with an all-pass window)
                    nc.vector._custom_dve(
                        TENSOR_MASK_REDUCE,
                        out=rb[:],
                        in0=ps[:],
                        in1=c3big[:],
                        s0=0.0,
                        s1=init,
                        imm2=1.0,
                        accum_out=pm[:, h:h + 1],
                    )
                    rbs.append(rb)

                # per-row constants: a' = rx5 / (1.001 - 5*rx5*pmax), bias = -a'*pmax
                smax = small.tile([128, 1], F32, tag="smax")
                nc.vector.tensor_scalar(
                    smax[:], pm[:, 1:2], rx5[:, k:k + 1], H_BAND, OP.mult, OP.mult
                )
                den = small.tile([128, 1], F32, tag="den")
                nc.vector.tensor_scalar(
                    den[:], smax[:], -1.0, 1.0 + EPS_MIN, OP.mult, OP.add
                )
                rec = small.tile([128, 1], F32, tag="rec")
                nc.vector.reciprocal(rec[:], den[:])
                aa = small.tile([128, 1], F32, tag="aa")
                nc.vector.tensor_scalar(aa[:], rec[:], rx5[:, k:k + 1], None, OP.mult)
                bb = small.tile([128, 1], F32, tag="bb")
                nc.vector.tensor_scalar(
                    bb[:], aa[:], pm[:, 1:2], -1.0, OP.mult, OP.mult
                )

                for h in range(2):
                    es = escrpool.tile([128, HALF], F32, tag="escr")
                    nc.scalar.activation(
                        es[:],
                        rbs[h][:],
                        AF.Exp,
                        bias=bb[:],
                        scale=aa[:],
                        accum_out=ssums[:, 2 * k + h:2 * k + h + 1],
                    )

            # ---------------- finalize ----------------
            sfin = persist.tile([128, CH], F32)
            nc.vector.reduce_sum(
                out=sfin[:],
                in_=ssums[:].rearrange("p (k t) -> p k t", t=2),
                axis=AxisListType.X,
            )
            nc.sync.dma_start(out=out_d[:], in_=sfin[:])

    nc.compile()
    return nc


def _get_nc():
    if "nc" not in _NC_CACHE:
        _NC_CACHE["nc"] = build_nc()
    return _NC_CACHE["nc"]


def make_in_maps(X_features, Y_features):
    X = np.ascontiguousarray(np.asarray(X_features, np.float32).reshape(B, C, N))
    Y = np.ascontiguousarray(np.asarray(Y_features, np.float32).reshape(B, C, N))
    in_maps = []
    for c in range(8):
        b, h = divmod(c, 2)
        in_maps.append({
            "Xh": np.ascontiguousarray(X[b, :, h * NX:(h + 1) * NX]),
            "Yb": Y[b],
        })
    return in_maps


def combine(results):
    """results: list of 8 dicts with 'out' [128, CH] = S' per row."""
    out = np.empty(B, np.float32)
    for b in range(B):
        tot = 0.0
        for h in range(2):
            s = results[2 * b + h]["out"].astype(np.float64)
            tot += (1.0 / s).sum()
        out[b] = -np.log(tot / N)
    return out


def kernel(X_features, Y_features):
    nc = _get_nc()
    in_maps = make_in_maps(X_features, Y_features)
    res = run_bass_kernel_spmd(nc, in_maps, core_ids=list(range(8)))
    return combine(res.results)


if __name__ == "__main__":
    rng = np.random.default_rng(0)
    X = rng.standard_normal((B, C, 64, 64)).astype(np.float32)
    Y = rng.standard_normal((B, C, 64, 64)).astype(np.float32)
    print(kernel(X_features=X, Y_features=Y))



# revision 8
# speedup vs baseline: 1.0359x; 1.0359x over previous
"""Trainium2 Bass kernel for a contextual loss (cosine-distance softmin loss).

Math (per batch b):
  mu_c      = mean_n Y[b,c,n]
  xc = X-mu, yc = Y-mu                      (centered, [C,N])
  psi[i,j]  = <xc_i, yc_j * ry_j>           (ry = 1/||yc_j||; f32r matmul)
  pm_i      = max_j psi[i,j]
  aa_i      = rx5_i / (1+EPS - 5*rx5_i*pm_i)    (rx5 = 0.2/||xc_i||)
  S_i       = sum_j exp(aa_i*(psi[i,j] - pm_i))
  CX_i      = 1/S_i ;  loss_b = -log(mean_i CX_i)

Sharding: 8 cores = 4 batches x 2 row-halves. Each core gets its
full-batch Y [64,4096] and its half of X's columns [64,2048], returns
S as [128,16] (partition p, chunk k <-> row k*128+p). Host reduces to
the [4] loss.

Structure notes:
 - ||yc_j||^2 is computed WITHOUT materializing yc: ny2 = ones@(Y^2)
   - 2mu@Y + |mu|^2 via two accumulating matmuls, so the Y^2 squares
   run during the input DMA instead of after the mean is known.
 - yhat = (Y - mu)*ry in one fused AFFINE_MUL_REDUCE DVE op.
 - per chunk: PE 8 f32r matmuls -> PSUM [128,2048] x2; DVE
   TENSOR_MASK_REDUCE copies PSUM->SBUF rowbuf with running row-max;
   ACT does ONE exp over [128,4096] with per-row scale/bias and
   accumulated row-sum. The aa/bb per-row constants are computed for
   PAIRS of chunks to halve small-op count on the (bottleneck) DVE.
"""

import math

import numpy as np

import concourse.bacc as bacc
import concourse.mybir as mybir
from concourse.dve_ops import AFFINE_MUL_REDUCE, TENSOR_MASK_REDUCE
from concourse.bass_utils import run_bass_kernel_spmd
from concourse.mybir import ActivationFunctionType as AF, AluOpType as OP, AxisListType
from concourse.tile import TileContext

F32 = mybir.dt.float32
F32R = mybir.dt.float32r
BF16 = mybir.dt.bfloat16

B, C, N = 4, 64, 4096          # batch, channels, spatial (64*64)
NX = N // 2                    # rows per core (half batch)
CH = NX // 128                 # 16 chunks of 128 rows
HALF = N // 2                  # column half per PSUM tile
H_BAND = 5.0
EPS_MIN = 1e-3
LN02 = math.log(0.2)

_NC_CACHE = {}


def build_nc():
    nc = bacc.Bacc("TRN2", target_bir_lowering=False, debug=False, num_devices=8)
    x_d = nc.dram_tensor("Xh", [C, NX], F32, kind="ExternalInput")
    y_d = nc.dram_tensor("Yb", [C, N], F32R, kind="ExternalInput")
    out_d = nc.dram_tensor("out", [128, CH], F32, kind="ExternalOutput")

    with TileContext(nc) as tc:
        with (
            tc.tile_pool(name="persist", bufs=1) as persist,
            tc.tile_pool(name="mm", bufs=2, space="PSUM") as mmpool,
            tc.tile_pool(name="rb", bufs=4) as rbpool,
            tc.tile_pool(name="es", bufs=2) as espool,
            tc.tile_pool(name="small", bufs=4) as small,
        ):
            # ---------------- load inputs (chunked for overlap) ----------------
            y_sb = persist.tile([C, N], F32R)
            for q in range(4):
                nc.sync.dma_start(
                    out=y_sb[:, q * 1024:(q + 1) * 1024],
                    in_=y_d[:, q * 1024:(q + 1) * 1024],
                )
            x_sb = persist.tile([C, NX], F32)
            for q in range(2):
                nc.sync.dma_start(
                    out=x_sb[:, q * 1024:(q + 1) * 1024],
                    in_=x_d[:, q * 1024:(q + 1) * 1024],
                )

            ones_f = persist.tile([C, 64], F32)
            nc.vector.memset(ones_f[:], 1.0)
            ones_w = persist.tile([C, 64], F32R)
            nc.vector.tensor_copy(ones_w[:], ones_f[:])
            ones1 = persist.tile([C, 2], F32R)
            nc.vector.tensor_copy(ones1[:], ones_f[:, 0:2])
            c3big = persist.tile([128, 1], F32)
            nc.vector.memset(c3big[:], 1.0e9)
            ln02 = persist.tile([128, 1], F32)
            nc.vector.memset(ln02[:], LN02)

            # squares of RAW Y overlap the DMA (no dependency on mu)
            ysq = persist.tile([C, N], F32R)
            for q in range(4):
                nc.scalar.activation(
                    ysq[:, q * 1024:(q + 1) * 1024],
                    y_sb[:, q * 1024:(q + 1) * 1024],
                    AF.Square,
                )

            # mean of Y (partial sums overlap the DMA)
            muparts = small.tile([C, 4], F32, tag="muparts")
            for q in range(4):
                nc.vector.reduce_sum(
                    out=muparts[:, q:q + 1],
                    in_=y_sb[:, q * 1024:(q + 1) * 1024].bitcast(F32),
                    axis=AxisListType.X,
                )
            musum = small.tile([C, 1], F32, tag="musum")
            nc.vector.reduce_sum(out=musum[:], in_=muparts[:], axis=AxisListType.X)
            mu = small.tile([C, 1], F32, tag="mu")
            nc.vector.tensor_scalar_mul(mu[:], musum[:], 1.0 / N)
            muneg = small.tile([C, 1], F32, tag="muneg")
            nc.vector.tensor_scalar_mul(muneg[:], mu[:], -1.0)
            mu2neg = small.tile([C, 1], F32, tag="mu2neg")
            nc.vector.tensor_scalar_mul(mu2neg[:], mu[:], -2.0)

            # lhsT for the -2*mu@Y correction and for w = |mu|^2 broadcast
            lhs_mu = persist.tile([C, 64], F32R)
            nc.vector.tensor_scalar(lhs_mu[:], ones_f[:], mu2neg[:], None, OP.mult)
            musq = small.tile([C, 1], F32, tag="musq")
            nc.vector.tensor_tensor(musq[:], mu[:], mu[:], OP.mult)
            lhs_w = persist.tile([C, 64], F32R)
            nc.vector.tensor_scalar(lhs_w[:], ones_f[:], musq[:], None, OP.mult)

            # w = |mu|^2 on 64 partitions (via ones matmul), to SBUF for Ln bias
            ps_w = mmpool.tile([128, HALF], F32, tag="mm")
            nc.tensor.matmul(
                ps_w[0:64, 0:2], lhsT=lhs_w[:], rhs=ones1[:], start=True, stop=True
            )
            w_sb = small.tile([C, 1], F32, tag="w")
            nc.vector.tensor_copy(w_sb[:], ps_w[0:64, 0:1])

            # ---------------- X side: center, norms, rx5 ----------------------
            xcen = persist.tile([C, NX], F32R)
            nc.vector.tensor_scalar(xcen[:], x_sb[:], mu[:], None, OP.subtract)
            xsq = persist.tile([C, NX], F32R)
            nc.scalar.activation(xsq[:], xcen[:], AF.Square)
            nx2 = mmpool.tile([128, HALF], F32, tag="mm")
            for k in range(CH):
                nc.tensor.matmul(
                    nx2[:, 2 * k:2 * k + 2],
                    lhsT=xsq[:, k * 128:(k + 1) * 128],
                    rhs=ones1[:],
                    start=True, stop=True,
                )
            tn = small.tile([128, CH], F32, tag="tn")
            nc.scalar.activation(
                tn[:], nx2[:, 0:2 * CH].rearrange("p (k two) -> p k two", two=2)[:, :, 0],
                AF.Ln,
            )
            # rx5 = 0.2/||xc||, qneg = -5*rx5 (so den = 1+eps - rx*pm)
            rx5 = persist.tile([128, CH], F32)
            nc.scalar.activation(rx5[:], tn[:], AF.Exp, bias=ln02[:], scale=-0.5)
            qneg = persist.tile([128, CH], F32)
            nc.vector.tensor_scalar_mul(qneg[:], rx5[:], -H_BAND)

            # ---------------- Y norms + yhat per column block -----------------
            # ny2 = ones@(Y^2) - 2mu@Y (accumulated), then +w inside Ln's bias;
            # ry = exp(-0.5*ln(ny2+w)); yhat = (Y - mu)*ry in one fused op.
            yhat = persist.tile([C, N], F32R)
            for b in range(2):
                sl = slice(b * HALF, (b + 1) * HALF)
                ny2 = mmpool.tile([128, HALF], F32, tag="mm")
                for j in range(4):
                    js = slice(b * HALF + j * 512, b * HALF + (j + 1) * 512)
                    pj = slice(j * 512, (j + 1) * 512)
                    nc.tensor.matmul(
                        ny2[0:64, pj], lhsT=ones_w[:], rhs=ysq[:, js],
                        start=True, stop=False,
                    )
                    nc.tensor.matmul(
                        ny2[0:64, pj], lhsT=lhs_mu[:],
                        rhs=y_sb[:, js],
                        start=False, stop=True,
                    )
                tln = espool.tile([128, HALF], F32, tag="tln")
                nc.scalar.activation(tln[0:64, :], ny2[0:64, :], AF.Ln, bias=w_sb[:])
                ry = rbpool.tile([C, HALF], F32, tag="ry")
                nc.scalar.activation(ry[:], tln[0:64, :], AF.Exp, scale=-0.5)
                nc.vector._custom_dve(
                    AFFINE_MUL_REDUCE,
                    out=yhat[:, sl],
                    in0=y_sb[:, sl].bitcast(F32),
                    in1=ry[:],
                    s0=1.0,
                    s1=muneg[:],
                )

            # ---------------- main loop ----------------
            pmall = persist.tile([128, CH], F32)
            aall = persist.tile([128, CH], F32)
            ball = persist.tile([128, CH], F32)
            ssums = persist.tile([128, CH], F32)
            rowbufs = {}
            for k in range(CH):
                lhs = xcen[:, k * 128:(k + 1) * 128]
                rowbuf = rbpool.tile([128, N], F32, tag="rb")
                rowbufs[k] = rowbuf
                for h in range(2):
                    ps = mmpool.tile([128, HALF], F32, tag="mm")
                    for j in range(4):
                        c0 = h * HALF + j * 512
                        nc.tensor.matmul(
                            ps[:, j * 512:(j + 1) * 512],
                            lhsT=lhs,
                            rhs=yhat[:, c0:c0 + 512],
                            start=True, stop=True,
                        )
                    init = -3.0e38 if h == 0 else pmall[:, k:k + 1]
                    # rowbuf half = copy(ps); pmall[:,k] = max(rowmax, init)
                    nc.vector._custom_dve(
                        TENSOR_MASK_REDUCE,
                        out=rowbuf[:, h * HALF:(h + 1) * HALF],
                        in0=ps[:],
                        in1=c3big[:],
                        s0=0.0,
                        s1=init,
                        imm2=1.0,
                        accum_out=pmall[:, k:k + 1],
                    )

                if k % 2 == 1:
                    # per-row constants for chunks k-1,k as one [128,2] batch:
                    # aa = rx5/(1+eps - 5*rx5*pm), bb = -aa*pm
                    pr = slice(k - 1, k + 1)
                    t2 = small.tile([128, 2], F32, tag="t2")
                    nc.vector.tensor_tensor(t2[:], pmall[:, pr], qneg[:, pr], OP.mult)
                    den = small.tile([128, 2], F32, tag="den")
                    nc.vector.tensor_scalar(
                        den[:], t2[:], 1.0 + EPS_MIN, None, OP.add
                    )
                    rec = small.tile([128, 2], F32, tag="rec")
                    nc.vector.reciprocal_approx_fast(rec[:], den[:])
                    nc.vector.tensor_tensor(aall[:, pr], rec[:], rx5[:, pr], OP.mult)
                    t3 = small.tile([128, 2], F32, tag="t3")
                    nc.vector.tensor_tensor(t3[:], aall[:, pr], pmall[:, pr], OP.mult)
                    nc.vector.tensor_scalar_mul(ball[:, pr], t3[:], -1.0)

                    for kk in (k - 1, k):
                        es = espool.tile([128, N], BF16, tag="es")
                        nc.scalar.activation(
                            es[:],
                            rowbufs.pop(kk)[:],
                            AF.Exp,
                            scale=aall[:, kk:kk + 1],
                            bias=ball[:, kk:kk + 1],
                            accum_out=ssums[:, kk:kk + 1],
                        )

            nc.sync.dma_start(out=out_d[:], in_=ssums[:])

    nc.compile()
    return nc


def _get_nc():
    if "nc" not in _NC_CACHE:
        _NC_CACHE["nc"] = build_nc()
    return _NC_CACHE["nc"]


def make_in_maps(X_features, Y_features):
    X = np.ascontiguousarray(np.asarray(X_features, np.float32).reshape(B, C, N))
    Y = np.ascontiguousarray(np.asarray(Y_features, np.float32).reshape(B, C, N))
    in_maps = []
    for c in range(8):
        b, h = divmod(c, 2)
        in_maps.append({
            "Xh": np.ascontiguousarray(X[b, :, h * NX:(h + 1) * NX]),
            "Yb": Y[b],
        })
    return in_maps


def combine(results):
    """results: list of 8 dicts with 'out' [128, CH] = S per row."""
    out = np.empty(B, np.float32)
    for b in range(B):
        tot = 0.0
        for h in range(2):
            s = results[2 * b + h]["out"].astype(np.float64)
            tot += (1.0 / s).sum()
        out[b] = -np.log(tot / N)
    return out


def kernel(X_features, Y_features):
    nc = _get_nc()
    in_maps = make_in_maps(X_features, Y_features)
    res = run_bass_kernel_spmd(nc, in_maps, core_ids=list(range(8)))
    return combine(res.results)


if __name__ == "__main__":
    rng = np.random.default_rng(0)
    X = rng.standard_normal((B, C, 64, 64)).astype(np.float32)
    Y = rng.standard_normal((B, C, 64, 64)).astype(np.float32)
    print(kernel(X_features=X, Y_features=Y))
